# revision 5
# baseline (speedup 1.0000x reference)
"""GCN message-passing kernel for 8 Trainium2 NeuronCores (Bass/Tile), v2.

Strategy (v2 — all-bf16 dataflow, chunked AllGather, lean PE):
  - Nodes sharded contiguously across 8 cores; edges dst-sorted into
    128-node window tiles (K_w shared across cores so the program is SPMD).
  - xl (node state after W-matmul + bias) stored bf16; AllGather'd in 4
    window-group chunks so the collective overlaps the previous layer's
    window loop.  xl_full uses a chunk-block layout [chunk][core][rows].
  - Per edge chunk (kg tiles): per-tile indirect row gather of xl[src]
    (bf16, 256B rows), host-precomputed per-edge bond embeddings DMA'd in,
    msg = relu(gather + ee) on vector, norm folded into the scatter one-hot
    (sel = is_equal(dstl, iota) * norm, built on vector in bf16).
  - Scatter-add per window via one-hot matmul into PSUM (bf16 operands).
  - Self term relu(xl)*deg_inv on scalar; h -> hT via DMA transpose (XBAR);
    BN affine + relu on scalar feature-major; next xl matmul in bf16 with
    W stationary slice; bias via 1-row ones matmul.
  - Encoder: host-precomputed atom-encoder output h0 uploaded feature-major;
    layer-0 xl is a single matmul per window.
  - Head/pool: bf16 head matmul + graph one-hot pool matmul per window;
    host combines per-window pooled blocks (+BN-shift/bias fixup).
"""

import numpy as np

import concourse.bass as bass
import concourse.bacc as bacc
import concourse.tile as tile
from concourse import mybir
from concourse.bass import IndirectOffsetOnAxis
from concourse.bass_utils import run_bass_kernel_spmd

F32 = mybir.dt.float32
BF16 = mybir.dt.bfloat16
I32 = mybir.dt.int32
BF16_NP = mybir.dt.np(BF16)

AF = mybir.ActivationFunctionType
ALU = mybir.AluOpType

# ----- problem constants (hardcoded; must match reference.py) -----
N_NODES = 200000
N_EDGES = 600000
N_GRAPHS = 4000
EMB = 128
LAYERS = 5
TASKS = 128
ATOM_FEATS, ATOM_VOCAB = 9, 64
BOND_FEATS, BOND_VOCAB = 3, 8
BN_EPS = 1e-5
N_CORES = 8
P = 128

CH_WINS = [56, 56, 56, 28]   # windows per AllGather chunk
GROUP = 7                    # windows per xlo/h0T/xls DMA group
KG = 16                      # tiles per gather/vector chunk


def _ceil_to(x, m):
    return (x + m - 1) // m * m


class Plan:
    """Host-side preprocessing: sharding, edge tiling, stream layouts."""

    def __init__(self, inputs, kg=KG):
        self.kg = kg
        x = np.asarray(inputs["x"])
        edge_index = np.asarray(inputs["edge_index"])
        edge_attr = np.asarray(inputs["edge_attr"])
        batch = np.asarray(inputs["batch"])
        n, n_cores = N_NODES, N_CORES
        sh = n // n_cores
        self.sh = sh
        nw = _ceil_to(sh, P) // P
        self.nw = nw
        npad = nw * P
        self.npad = npad
        assert sum(CH_WINS) == nw and nw % GROUP == 0

        self.rq = [w * P for w in CH_WINS]
        self.cum = np.concatenate([[0], np.cumsum(self.rq)]).astype(np.int64)
        self.blk = [8 * int(c) for c in self.cum]

        src = edge_index[0].astype(np.int64)
        dst = edge_index[1].astype(np.int64)

        deg = (np.bincount(src, minlength=n).astype(np.float32) + 1.0)
        dinv_sqrt = deg ** -0.5
        norm_e = (dinv_sqrt[src] * dinv_sqrt[dst]).astype(np.float32)
        self.deg_inv = (1.0 / deg).astype(np.float32)

        order = np.argsort(dst, kind="stable")
        src_s, dst_s, norm_s = src[order], dst[order], norm_e[order]
        attr_s = edge_attr[order].astype(np.int64)

        core_of = dst_s // sh
        win_of = (dst_s % sh) // P
        counts = np.zeros((n_cores, nw), dtype=np.int64)
        np.add.at(counts, (core_of, win_of), 1)
        K_w = np.maximum(np.ceil(counts.max(axis=0) / P).astype(np.int64), 1)
        self.K_w = K_w.tolist()
        T = int(K_w.sum())
        self.T = _ceil_to(T, kg)
        self.pad_tiles = self.T - T

        cum, blk, rq = self.cum, self.blk, self.rq

        def gpos(nodes):
            r = nodes // sh
            p = nodes % sh
            q = np.minimum(np.searchsorted(cum, p, side="right") - 1, 3)
            off = p - cum[q]
            rqa = np.asarray(rq)[q]
            return (np.asarray(blk)[q] + r * rqa + off).astype(np.int32)

        E_pad = self.T * P
        self.src_pos = np.zeros((n_cores, P, self.T), dtype=np.int32)
        self.combo = np.zeros((n_cores, self.T * P), dtype=np.int64)
        self.norm_st = np.zeros((n_cores, P, self.T), dtype=BF16_NP)
        self.dstl_st = np.full((n_cores, P, self.T), -1.0, dtype=BF16_NP)

        for c in range(n_cores):
            m = core_of == c
            e_src, e_dst, e_nrm, e_att = src_s[m], dst_s[m], norm_s[m], attr_s[m]
            e_win = (e_dst % sh) // P
            stream_src = np.zeros(E_pad, dtype=np.int32)
            stream_nrm = np.zeros(E_pad, dtype=np.float32)
            stream_dstl = np.full(E_pad, -1.0, dtype=np.float32)
            stream_cmb = np.zeros(E_pad, dtype=np.int64)
            base = 0
            for w, k in enumerate(self.K_w):
                s = e_win == w
                cnt = int(s.sum())
                assert cnt <= k * P
                sl = slice(base, base + cnt)
                stream_src[sl] = gpos(e_src[s])
                stream_nrm[sl] = e_nrm[s]
                stream_dstl[sl] = (e_dst[s] % sh - w * P).astype(np.float32)
                att = e_att[s]
                stream_cmb[sl] = att[:, 0] * 64 + att[:, 1] * 8 + att[:, 2]
                base += k * P
            self.src_pos[c] = stream_src.reshape(self.T, P).T
            self.norm_st[c] = stream_nrm.reshape(self.T, P).T.astype(BF16_NP)
            self.dstl_st[c] = stream_dstl.reshape(self.T, P).T.astype(BF16_NP)
            self.combo[c] = stream_cmb

        div = np.zeros((n_cores, P, nw), dtype=np.float32)
        for c in range(n_cores):
            d = self.deg_inv[c * sh:(c + 1) * sh]
            d = np.pad(d, (0, npad - sh))
            div[c] = d.reshape(nw, P).T
        self.dinv_w = div

        # pooling structures
        self.glocal = np.full((n_cores, P, nw), -1.0, dtype=BF16_NP)
        self.gmap = np.zeros((n_cores, nw, P), dtype=np.int64)
        self.gmap_n = np.zeros((n_cores, nw), dtype=np.int64)
        for c in range(n_cores):
            b = batch[c * sh:(c + 1) * sh]
            for w in range(nw):
                bw = b[w * P:(w + 1) * P]
                if len(bw) == 0:
                    continue
                uniq, inv = np.unique(np.asarray(bw), return_inverse=True)
                assert len(uniq) <= P
                gl = np.full(P, -1.0, dtype=np.float32)
                gl[:len(bw)] = inv.astype(np.float32)
                self.glocal[c, :, w] = gl.astype(BF16_NP)
                self.gmap[c, w, :len(uniq)] = uniq
                self.gmap_n[c, w] = len(uniq)

        self.cnt_g = np.bincount(np.asarray(batch), minlength=N_GRAPHS
                                 ).astype(np.float32)
        self.x = x

    def weight_arrays(self, inputs):
        atom_emb = np.asarray(inputs["atom_emb"], np.float32)
        bond_emb = np.asarray(inputs["bond_emb"], np.float32)
        W = np.asarray(inputs["W"], np.float32)
        b = np.asarray(inputs["b"], np.float32)
        root = np.asarray(inputs["root"], np.float32)
        bn_mean = np.asarray(inputs["bn_mean"], np.float32)
        bn_var = np.asarray(inputs["bn_var"], np.float32)
        bn_gamma = np.asarray(inputs["bn_gamma"], np.float32)
        bn_beta = np.asarray(inputs["bn_beta"], np.float32)
        headW = np.asarray(inputs["headW"], np.float32)
        headb = np.asarray(inputs["headb"], np.float32)

        out = {}
        # atom-encoder output, per-core feature-major bf16 [P, npad]
        x = self.x
        h0 = np.zeros((N_NODES, EMB), np.float32)
        for f in range(ATOM_FEATS):
            h0 += atom_emb[f][np.asarray(x[:, f])]
        h0T = np.zeros((N_CORES, P, self.npad), dtype=BF16_NP)
        for c in range(N_CORES):
            hc = h0[c * self.sh:(c + 1) * self.sh]
            hc = np.pad(hc, ((0, self.npad - self.sh), (0, 0)))
            h0T[c] = hc.T.astype(BF16_NP)
        self.h0T = h0T

        out["Wlb"] = W.transpose(1, 0, 2).reshape(EMB, LAYERS * EMB) \
            .astype(BF16_NP)
        out["rootb"] = (root + b).reshape(1, LAYERS * EMB).astype(BF16_NP)

        # per-edge ee streams: ee = sum_f bond_emb[l,f,attr_f] - root_l,
        # laid out to match gather tiles: [L, P, T*EMB] (partition = edge%P)
        a0, a1, a2 = np.meshgrid(np.arange(8), np.arange(8), np.arange(8),
                                 indexing="ij")
        eetabs = np.zeros((LAYERS, 512, EMB), np.float32)
        for l in range(LAYERS):
            eetabs[l] = (bond_emb[l, 0][a0.ravel()]
                         + bond_emb[l, 1][a1.ravel()]
                         + bond_emb[l, 2][a2.ravel()] - root[l][None, :])
        self.eetabs = eetabs  # used by build_ee per core

        s = (bn_gamma / np.sqrt(bn_var + BN_EPS)).astype(np.float32)
        t = (bn_beta - bn_mean * s).astype(np.float32)
        out["bnS"] = s.T.copy()
        out["bnB"] = t.T.copy()
        out["headWp"] = (s[LAYERS - 1][:, None] * headW).astype(BF16_NP)
        self.crow = (t[LAYERS - 1] @ headW).astype(np.float32)
        self.headb = headb
        out["iota"] = np.tile(np.arange(P, dtype=np.float32), (P, 1)) \
            .astype(BF16_NP)
        out["ones1"] = np.ones((1, P), dtype=BF16_NP)
        return out

    def build_ee(self, c):
        """Per-core per-edge ee stream [LAYERS, P, T*EMB] bf16."""
        T = self.T
        ee = np.zeros((LAYERS, P, T * EMB), dtype=BF16_NP)
        cmb = self.combo[c]
        for l in range(LAYERS):
            v = self.eetabs[l][cmb]                       # [T*P, EMB]
            v = v.reshape(T, P, EMB).transpose(1, 0, 2)   # [P, T, EMB]
            ee[l] = v.reshape(P, T * EMB).astype(BF16_NP)
        return ee

    def postprocess(self, pooled_blocks):
        out = np.zeros((N_GRAPHS, TASKS), dtype=np.float32)
        for c in range(N_CORES):
            blk = pooled_blocks[c]
            for w in range(self.nw):
                k = int(self.gmap_n[c, w])
                if k:
                    np.add.at(out, self.gmap[c, w, :k], blk[w, :k])
        out += self.cnt_g[:, None] * self.crow[None, :] + self.headb[None, :]
        return out


def build_program(plan):
    nc = bacc.Bacc(None, target_bir_lowering=False, debug=False)
    nw, T, kg = plan.nw, plan.T, plan.kg
    npad = plan.npad
    n_cores = N_CORES
    ngrp = nw // GROUP          # window groups
    grp_per_chunk = [w // GROUP for w in CH_WINS]   # groups per AG chunk

    def par(name, shape, dt):
        return nc.declare_dram_parameter(name, list(shape), dt, isOutput=False)

    p_h0T = par("h0T", (P, npad), BF16)
    p_W = par("Wlb", (EMB, LAYERS * EMB), BF16)
    p_rootb = par("rootb", (1, LAYERS * EMB), BF16)
    p_ee = par("ee_all", (LAYERS, P, T * EMB), BF16)
    p_bnS = par("bnS", (EMB, LAYERS), F32)
    p_bnB = par("bnB", (EMB, LAYERS), F32)
    p_headW = par("headWp", (EMB, TASKS), BF16)
    p_iota = par("iota", (P, P), BF16)
    p_ones1 = par("ones1", (1, P), BF16)
    p_src = par("src_pos", (P, T), I32)
    p_norm = par("norm_st", (P, T), BF16)
    p_dstl = par("dstl_st", (P, T), BF16)
    p_dinv = par("dinv_w", (P, nw), F32)
    p_gloc = par("glocal", (P, nw), BF16)
    p_out = nc.declare_dram_parameter("out", [nw, P, TASKS], F32, isOutput=True)

    xl_sh = [nc.dram_tensor(f"xl_sh{i}", [npad, EMB], BF16) for i in range(2)]
    xl_full = [nc.dram_tensor(f"xl_full{i}", [n_cores * npad, EMB], BF16,
                              addr_space="Shared") for i in range(2)]

    groups = [list(range(n_cores))]
    cum, blk = plan.cum, plan.blk

    with tile.TileContext(nc) as tc:
        with tc.tile_pool(name="const", bufs=1) as cpool, \
             tc.tile_pool(name="sbw", bufs=3) as sbw, \
             tc.tile_pool(name="sbg", bufs=2) as sbg, \
             tc.tile_pool(name="sbx", bufs=2) as sbx, \
             tc.tile_pool(name="psA", bufs=3, space="PSUM") as psA, \
             tc.tile_pool(name="psM", bufs=3, space="PSUM") as psM:

            def cload(ap, shape, dt, name):
                t = cpool.tile(list(shape), dt, tag=name)
                nc.sync.dma_start(out=t[:], in_=ap)
                return t

            iota = cload(p_iota[:, :], (P, P), BF16, "iota")
            ones1 = cload(p_ones1[:, :], (1, P), BF16, "ones1")
            Wl = cload(p_W[:, :], (EMB, LAYERS * EMB), BF16, "Wl")
            rootb = cload(p_rootb[:, :], (1, LAYERS * EMB), BF16, "rootb")
            bnS = cload(p_bnS[:, :], (EMB, LAYERS), F32, "bnS")
            bnB = cload(p_bnB[:, :], (EMB, LAYERS), F32, "bnB")
            headW = cload(p_headW[:, :], (EMB, TASKS), BF16, "headW")
            srcs = cload(p_src[:, :], (P, T), I32, "srcs")
            norms = cload(p_norm[:, :], (P, T), BF16, "norms")
            dstls = cload(p_dstl[:, :], (P, T), BF16, "dstls")
            dinvw = cload(p_dinv[:, :], (P, nw), F32, "dinvw")
            glocw = cload(p_gloc[:, :], (P, nw), BF16, "glocw")

            def emit_ag(l, q):
                """AllGather chunk q of xl for layer l (reads xl_sh[l%2])."""
                r0, r1 = int(cum[q]), int(cum[q + 1])
                nc.gpsimd.collective_compute(
                    "AllGather", ALU.bypass,
                    ins=[xl_sh[l % 2][r0:r1, :].opt()],
                    outs=[xl_full[l % 2][blk[q]:blk[q + 1], :].opt()],
                    replica_groups=groups)

            def xl_tail(l, w, hTs, xlsg):
                """hTs [feat,nodes] -> xl for layer l+1 into xlsg col w%GROUP."""
                xlp = psM.tile([P, EMB], F32, tag="mm")
                nc.tensor.matmul(out=xlp[:], lhsT=hTs[:],
                                 rhs=Wl[:, (l + 1) * EMB:(l + 2) * EMB],
                                 start=True, stop=False)
                nc.tensor.matmul(out=xlp[:], lhsT=ones1[:],
                                 rhs=rootb[0:1, (l + 1) * EMB:(l + 2) * EMB],
                                 start=False, stop=True)
                i = w % GROUP
                nc.vector.tensor_copy(out=xlsg[:, i * EMB:(i + 1) * EMB],
                                      in_=xlp[:])

            def head_tail(l, w, hTs):
                """Last layer: head matmul + graph pooling, DMA to p_out."""
                zp = psM.tile([P, TASKS], F32, tag="mm")
                nc.tensor.matmul(out=zp[:], lhsT=hTs[:], rhs=headW[:],
                                 start=True, stop=True)
                zs = sbw.tile([P, TASKS], BF16, tag="zs")
                nc.scalar.activation(out=zs[:], in_=zp[:], func=AF.Copy)
                selg = sbw.tile([P, P], BF16, tag="selg")
                nc.vector.tensor_tensor(
                    out=selg[:],
                    in0=glocw[:, w:w + 1].to_broadcast([P, P]),
                    in1=iota[:], op=ALU.is_equal)
                pp = psM.tile([P, TASKS], F32, tag="mm")
                nc.tensor.matmul(out=pp[:], lhsT=selg[:], rhs=zs[:],
                                 start=True, stop=True)
                ps = sbw.tile([P, TASKS], F32, tag="ps")
                nc.vector.tensor_copy(out=ps[:], in_=pp[:])
                nc.sync.dma_start(out=p_out[w, :, :], in_=ps[:])

            # ---------------- encoder: xl_0 ----------------
            for g in range(ngrp):
                h0g = sbx.tile([P, GROUP * EMB], BF16, tag="h0g")
                nc.sync.dma_start(
                    out=h0g[:], in_=p_h0T[:, g * GROUP * P:(g + 1) * GROUP * P])
                xlsg = sbx.tile([P, GROUP * EMB], BF16, tag="xlsg")
                for i in range(GROUP):
                    xlp = psM.tile([P, EMB], F32, tag="mm")
                    nc.tensor.matmul(out=xlp[:],
                                     lhsT=h0g[:, i * P:(i + 1) * P],
                                     rhs=Wl[:, 0:EMB], start=True, stop=False)
                    nc.tensor.matmul(out=xlp[:], lhsT=ones1[:],
                                     rhs=rootb[0:1, 0:EMB],
                                     start=False, stop=True)
                    nc.vector.tensor_copy(out=xlsg[:, i * EMB:(i + 1) * EMB],
                                          in_=xlp[:])
                nc.sync.dma_start(
                    out=xl_sh[0][g * GROUP * P:(g + 1) * GROUP * P, :]
                    .rearrange("(a p) e -> p a e", p=P),
                    in_=xlsg[:].rearrange("p (a e) -> p a e", e=EMB))
                if g + 1 in np.cumsum(grp_per_chunk).tolist():
                    emit_ag(0, int(np.searchsorted(
                        np.cumsum(grp_per_chunk), g + 1, side="left")))

            # ---------------- layers ----------------
            for l in range(LAYERS):
                xlf = xl_full[l % 2]
                cur = xl_sh[l % 2]
                gbuf = msgb = selb = None
                t_idx = 0

                def emit_chunk(j, l=l, xlf=xlf):
                    nonlocal gbuf, msgb, selb
                    t0 = j * kg
                    gbuf = sbg.tile([P, kg * EMB], BF16, tag="gbuf")
                    for i in range(kg):
                        nc.gpsimd.indirect_dma_start(
                            out=gbuf[:, i * EMB:(i + 1) * EMB],
                            out_offset=None,
                            in_=xlf[:, :],
                            in_offset=IndirectOffsetOnAxis(
                                ap=srcs[:, t0 + i:t0 + i + 1], axis=0))
                    eeb = sbg.tile([P, kg * EMB], BF16, tag="eeb")
                    nc.sync.dma_start(
                        out=eeb[:], in_=p_ee[l, :, t0 * EMB:(t0 + kg) * EMB])
                    selb = sbg.tile([P, kg * P], BF16, tag="selb")
                    nc.vector.tensor_tensor(
                        out=selb[:].rearrange("p (k c) -> p k c", k=kg),
                        in0=dstls[:, t0:t0 + kg].unsqueeze(2)
                        .to_broadcast([P, kg, P]),
                        in1=iota[:].unsqueeze(1).to_broadcast([P, kg, P]),
                        op=ALU.is_equal)
                    nc.vector.tensor_tensor(
                        out=selb[:].rearrange("p (k c) -> p k c", k=kg),
                        in0=selb[:].rearrange("p (k c) -> p k c", k=kg),
                        in1=norms[:, t0:t0 + kg].unsqueeze(2)
                        .to_broadcast([P, kg, P]),
                        op=ALU.mult)
                    msgb = sbg.tile([P, kg * EMB], BF16, tag="msgb")
                    nc.vector.tensor_tensor(out=msgb[:], in0=gbuf[:],
                                            in1=eeb[:], op=ALU.add)
                    nc.scalar.activation(out=msgb[:], in_=msgb[:],
                                         func=AF.Relu)

                for g in range(ngrp):
                    xlog = sbx.tile([P, GROUP * EMB], BF16, tag="xlog")
                    nc.sync.dma_start(
                        out=xlog[:].rearrange("p (a e) -> p a e", e=EMB),
                        in_=cur[g * GROUP * P:(g + 1) * GROUP * P, :]
                        .rearrange("(a p) e -> p a e", p=P))
                    xlsg = None
                    if l < LAYERS - 1:
                        xlsg = sbx.tile([P, GROUP * EMB], BF16, tag="xlsg")
                    for i in range(GROUP):
                        w = g * GROUP + i
                        aggp = psA.tile([P, EMB], F32, tag="agg")
                        kw = plan.K_w[w]
                        for k in range(kw):
                            t = t_idx + k
                            if t % kg == 0:
                                emit_chunk(t // kg)
                            base = (t % kg)
                            nc.tensor.matmul(
                                out=aggp[:],
                                lhsT=selb[:, base * P:(base + 1) * P],
                                rhs=msgb[:, base * EMB:(base + 1) * EMB],
                                start=(k == 0), stop=(k == kw - 1))
                        t_idx += kw

                        sf = sbw.tile([P, EMB], F32, tag="sf")
                        nc.scalar.activation(
                            out=sf[:], in_=xlog[:, i * EMB:(i + 1) * EMB],
                            func=AF.Relu, scale=dinvw[:, w:w + 1])
                        hnew = sbw.tile([P, EMB], BF16, tag="hnew")
                        nc.vector.tensor_tensor(out=hnew[:], in0=sf[:],
                                                in1=aggp[:], op=ALU.add)
                        hT = sbw.tile([P, EMB], BF16, tag="hT")
                        nc.sync.dma_start_transpose(hT[:], hnew[:])
                        hTs = sbw.tile([P, EMB], BF16, tag="hTs")
                        if l < LAYERS - 1:
                            nc.scalar.activation(
                                out=hTs[:], in_=hT[:], func=AF.Relu,
                                scale=bnS[:, l:l + 1], bias=bnB[:, l:l + 1])
                            xl_tail(l, w, hTs, xlsg)
                        else:
                            nc.scalar.activation(out=hTs[:], in_=hT[:],
                                                 func=AF.Copy)
                            head_tail(l, w, hTs)
                    if l < LAYERS - 1:
                        nxt = xl_sh[(l + 1) % 2]
                        nc.sync.dma_start(
                            out=nxt[g * GROUP * P:(g + 1) * GROUP * P, :]
                            .rearrange("(a p) e -> p a e", p=P),
                            in_=xlsg[:].rearrange("p (a e) -> p a e", e=EMB))
                        if g + 1 in np.cumsum(grp_per_chunk).tolist():
                            emit_ag(l + 1, int(np.searchsorted(
                                np.cumsum(grp_per_chunk), g + 1, side="left")))

    nc.finalize()
    return nc


_CACHE = {}


def kernel(**inputs):
    key = "prog"
    if key not in _CACHE:
        plan = Plan(inputs)
        warr = plan.weight_arrays(inputs)
        nc = build_program(plan)
        _CACHE[key] = (plan, nc)
    else:
        plan, nc = _CACHE[key]
        warr = plan.weight_arrays(inputs)

    in_maps = []
    for c in range(N_CORES):
        m = dict(warr)
        m["h0T"] = plan.h0T[c]
        m["ee_all"] = plan.build_ee(c)
        m["src_pos"] = plan.src_pos[c]
        m["norm_st"] = plan.norm_st[c]
        m["dstl_st"] = plan.dstl_st[c]
        m["dinv_w"] = plan.dinv_w[c]
        m["glocal"] = plan.glocal[c]
        in_maps.append(m)

    import os
    trace = bool(os.environ.get("BASS_GNN_TRACE"))
    if trace:
        try:
            import ntff_hook
            ntff_hook.install()
        except Exception:
            trace = False
    res = run_bass_kernel_spmd(nc, in_maps, list(range(N_CORES)),
                               trace=trace)
    global _LAST_EXEC_NS, _LAST_RES
    _LAST_EXEC_NS = res.exec_time_ns
    _LAST_RES = res
    blocks = [np.asarray(r["out"], np.float32) for r in res.results]
    return plan.postprocess(blocks)


# revision 14
# speedup vs baseline: 1.5315x; 1.5315x over previous
"""GCN message-passing kernel for 8 Trainium2 NeuronCores (Bass/Tile), v2.

Strategy (v2 — all-bf16 dataflow, chunked AllGather, lean PE):
  - Nodes sharded contiguously across 8 cores; edges dst-sorted into
    128-node window tiles (K_w shared across cores so the program is SPMD).
  - xl (node state after W-matmul + bias) stored bf16; AllGather'd in 4
    window-group chunks so the collective overlaps the previous layer's
    window loop.  xl_full uses a chunk-block layout [chunk][core][rows].
  - Per edge chunk (kg tiles): per-tile indirect row gather of xl[src]
    (bf16, 256B rows), host-precomputed per-edge bond embeddings DMA'd in,
    msg = relu(gather + ee) on vector, norm folded into the scatter one-hot
    (sel = is_equal(dstl, iota) * norm, built on vector in bf16).
  - Scatter-add per window via one-hot matmul into PSUM (bf16 operands).
  - Self term relu(xl)*deg_inv on scalar; h -> hT via DMA transpose (XBAR);
    BN affine + relu on scalar feature-major; next xl matmul in bf16 with
    W stationary slice; bias via 1-row ones matmul.
  - Encoder: host-precomputed atom-encoder output h0 uploaded feature-major;
    layer-0 xl is a single matmul per window.
  - Head/pool: bf16 head matmul + graph one-hot pool matmul per window;
    host combines per-window pooled blocks (+BN-shift/bias fixup).
"""

import numpy as np

import concourse.bass as bass
import concourse.bacc as bacc
import concourse.tile as tile
from concourse import mybir
from concourse.bass import IndirectOffsetOnAxis
from concourse.bass_utils import run_bass_kernel_spmd

F32 = mybir.dt.float32
BF16 = mybir.dt.bfloat16
I32 = mybir.dt.int32
BF16_NP = mybir.dt.np(BF16)

AF = mybir.ActivationFunctionType
ALU = mybir.AluOpType

# ----- problem constants (hardcoded; must match reference.py) -----
N_NODES = 200000
N_EDGES = 600000
N_GRAPHS = 4000
EMB = 128
LAYERS = 5
TASKS = 128
ATOM_FEATS, ATOM_VOCAB = 9, 64
BOND_FEATS, BOND_VOCAB = 3, 8
BN_EPS = 1e-5
N_CORES = 8
P = 128

CH_WINS = [56, 56, 56, 28]   # windows per AllGather chunk
GROUP = 7                    # windows per xlo/h0T/xls DMA group
KG = 16                      # tiles per gather/vector chunk


def _ceil_to(x, m):
    return (x + m - 1) // m * m


class Plan:
    """Host-side preprocessing: sharding, edge tiling, stream layouts."""

    def __init__(self, inputs, kg=KG):
        self.kg = kg
        x = np.asarray(inputs["x"])
        edge_index = np.asarray(inputs["edge_index"])
        edge_attr = np.asarray(inputs["edge_attr"])
        batch = np.asarray(inputs["batch"])
        n, n_cores = N_NODES, N_CORES
        sh = n // n_cores
        self.sh = sh
        nw = _ceil_to(sh, P) // P
        self.nw = nw
        npad = nw * P
        self.npad = npad
        assert sum(CH_WINS) == nw and nw % GROUP == 0

        self.rq = [w * P for w in CH_WINS]
        self.cum = np.concatenate([[0], np.cumsum(self.rq)]).astype(np.int64)
        self.blk = [8 * int(c) for c in self.cum]

        src = edge_index[0].astype(np.int64)
        dst = edge_index[1].astype(np.int64)

        deg = (np.bincount(src, minlength=n).astype(np.float32) + 1.0)
        dinv_sqrt = deg ** -0.5
        norm_e = (dinv_sqrt[src] * dinv_sqrt[dst]).astype(np.float32)
        self.deg_inv = (1.0 / deg).astype(np.float32)

        order = np.argsort(dst, kind="stable")
        src_s, dst_s, norm_s = src[order], dst[order], norm_e[order]
        attr_s = edge_attr[order].astype(np.int64)

        core_of = dst_s // sh
        win_of = (dst_s % sh) // P
        counts = np.zeros((n_cores, nw), dtype=np.int64)
        np.add.at(counts, (core_of, win_of), 1)
        K_w = np.maximum(np.ceil(counts.max(axis=0) / P).astype(np.int64), 1)
        self.K_w = K_w.tolist()
        T = int(K_w.sum())
        self.T = _ceil_to(T, kg)
        self.pad_tiles = self.T - T

        cum, blk, rq = self.cum, self.blk, self.rq

        def gpos(nodes):
            r = nodes // sh
            p = nodes % sh
            q = np.minimum(np.searchsorted(cum, p, side="right") - 1, 3)
            off = p - cum[q]
            rqa = np.asarray(rq)[q]
            return (np.asarray(blk)[q] + r * rqa + off).astype(np.int32)

        E_pad = self.T * P
        self.src_pos = np.zeros((n_cores, P, self.T), dtype=np.int32)
        self.combo = np.zeros((n_cores, self.T * P), dtype=np.int64)
        self.norm_st = np.zeros((n_cores, P, self.T), dtype=BF16_NP)
        self.dstl_st = np.full((n_cores, P, self.T), -1.0, dtype=BF16_NP)

        for c in range(n_cores):
            m = core_of == c
            e_src, e_dst, e_nrm, e_att = src_s[m], dst_s[m], norm_s[m], attr_s[m]
            e_win = (e_dst % sh) // P
            stream_src = np.zeros(E_pad, dtype=np.int32)
            stream_nrm = np.zeros(E_pad, dtype=np.float32)
            stream_dstl = np.full(E_pad, -1.0, dtype=np.float32)
            stream_cmb = np.zeros(E_pad, dtype=np.int64)
            base = 0
            for w, k in enumerate(self.K_w):
                s = e_win == w
                cnt = int(s.sum())
                assert cnt <= k * P
                sl = slice(base, base + cnt)
                stream_src[sl] = gpos(e_src[s])
                stream_nrm[sl] = e_nrm[s]
                stream_dstl[sl] = (e_dst[s] % sh - w * P).astype(np.float32)
                att = e_att[s]
                stream_cmb[sl] = att[:, 0] * 64 + att[:, 1] * 8 + att[:, 2]
                base += k * P
            self.src_pos[c] = stream_src.reshape(self.T, P).T
            self.norm_st[c] = stream_nrm.reshape(self.T, P).T.astype(BF16_NP)
            self.dstl_st[c] = stream_dstl.reshape(self.T, P).T.astype(BF16_NP)
            self.combo[c] = stream_cmb

        div = np.zeros((n_cores, P, nw), dtype=np.float32)
        for c in range(n_cores):
            d = self.deg_inv[c * sh:(c + 1) * sh]
            d = np.pad(d, (0, npad - sh))
            div[c] = d.reshape(nw, P).T
        self.dinv_w = div

        # pooling structures
        self.glocal = np.full((n_cores, P, nw), -1.0, dtype=BF16_NP)
        self.gmap = np.zeros((n_cores, nw, P), dtype=np.int64)
        self.gmap_n = np.zeros((n_cores, nw), dtype=np.int64)
        for c in range(n_cores):
            b = batch[c * sh:(c + 1) * sh]
            for w in range(nw):
                bw = b[w * P:(w + 1) * P]
                if len(bw) == 0:
                    continue
                uniq, inv = np.unique(np.asarray(bw), return_inverse=True)
                assert len(uniq) <= P
                gl = np.full(P, -1.0, dtype=np.float32)
                gl[:len(bw)] = inv.astype(np.float32)
                self.glocal[c, :, w] = gl.astype(BF16_NP)
                self.gmap[c, w, :len(uniq)] = uniq
                self.gmap_n[c, w] = len(uniq)

        self.cnt_g = np.bincount(np.asarray(batch), minlength=N_GRAPHS
                                 ).astype(np.float32)
        self.x = x

    def weight_arrays(self, inputs):
        atom_emb = np.asarray(inputs["atom_emb"], np.float32)
        bond_emb = np.asarray(inputs["bond_emb"], np.float32)
        W = np.asarray(inputs["W"], np.float32)
        b = np.asarray(inputs["b"], np.float32)
        root = np.asarray(inputs["root"], np.float32)
        bn_mean = np.asarray(inputs["bn_mean"], np.float32)
        bn_var = np.asarray(inputs["bn_var"], np.float32)
        bn_gamma = np.asarray(inputs["bn_gamma"], np.float32)
        bn_beta = np.asarray(inputs["bn_beta"], np.float32)
        headW = np.asarray(inputs["headW"], np.float32)
        headb = np.asarray(inputs["headb"], np.float32)

        out = {}
        # atom-encoder output, per-core feature-major bf16 [P, npad]
        x = self.x
        h0 = np.zeros((N_NODES, EMB), np.float32)
        for f in range(ATOM_FEATS):
            h0 += atom_emb[f][np.asarray(x[:, f])]
        h0T = np.zeros((N_CORES, P, self.npad), dtype=BF16_NP)
        for c in range(N_CORES):
            hc = h0[c * self.sh:(c + 1) * self.sh]
            hc = np.pad(hc, ((0, self.npad - self.sh), (0, 0)))
            h0T[c] = hc.T.astype(BF16_NP)
        self.h0T = h0T

        out["Wlb"] = W.transpose(1, 0, 2).reshape(EMB, LAYERS * EMB) \
            .astype(BF16_NP)
        out["rootb"] = (root + b).reshape(1, LAYERS * EMB).astype(BF16_NP)

        # per-edge ee streams: ee = sum_f bond_emb[l,f,attr_f] - root_l,
        # laid out to match gather tiles: [L, P, T*EMB] (partition = edge%P)
        a0, a1, a2 = np.meshgrid(np.arange(8), np.arange(8), np.arange(8),
                                 indexing="ij")
        eetabs = np.zeros((LAYERS, 512, EMB), np.float32)
        for l in range(LAYERS):
            eetabs[l] = (bond_emb[l, 0][a0.ravel()]
                         + bond_emb[l, 1][a1.ravel()]
                         + bond_emb[l, 2][a2.ravel()] - root[l][None, :])
        self.eetabs = eetabs  # used by build_ee per core

        s = (bn_gamma / np.sqrt(bn_var + BN_EPS)).astype(np.float32)
        t = (bn_beta - bn_mean * s).astype(np.float32)
        out["bnS"] = s.T.copy()
        out["bnB"] = t.T.copy()
        out["headWp"] = (s[LAYERS - 1][:, None] * headW).astype(BF16_NP)
        self.crow = (t[LAYERS - 1] @ headW).astype(np.float32)
        self.headb = headb
        out["iota"] = np.tile(np.arange(P, dtype=np.float32), (P, 1)) \
            .astype(BF16_NP)
        out["iden"] = np.eye(P, dtype=np.float32).astype(BF16_NP)
        out["ones1"] = np.ones((1, P), dtype=BF16_NP)
        return out

    def build_ee(self, c):
        """Per-core per-edge ee stream [LAYERS, P, T*EMB] bf16."""
        T = self.T
        ee = np.zeros((LAYERS, P, T * EMB), dtype=BF16_NP)
        cmb = self.combo[c]
        for l in range(LAYERS):
            v = self.eetabs[l][cmb]                       # [T*P, EMB]
            v = v.reshape(T, P, EMB).transpose(1, 0, 2)   # [P, T, EMB]
            ee[l] = v.reshape(P, T * EMB).astype(BF16_NP)
        return ee

    def postprocess(self, pooled_blocks):
        out = np.zeros((N_GRAPHS, TASKS), dtype=np.float32)
        for c in range(N_CORES):
            blk = pooled_blocks[c]
            for w in range(self.nw):
                k = int(self.gmap_n[c, w])
                if k:
                    np.add.at(out, self.gmap[c, w, :k], blk[w, :k])
        out += self.cnt_g[:, None] * self.crow[None, :] + self.headb[None, :]
        return out


def build_program(plan):
    nc = bacc.Bacc(None, target_bir_lowering=False, debug=False)
    nw, T, kg = plan.nw, plan.T, plan.kg
    npad = plan.npad
    n_cores = N_CORES
    ngrp = nw // GROUP          # window groups
    grp_per_chunk = [w // GROUP for w in CH_WINS]   # groups per AG chunk

    def par(name, shape, dt):
        return nc.declare_dram_parameter(name, list(shape), dt, isOutput=False)

    p_h0T = par("h0T", (P, npad), BF16)
    p_W = par("Wlb", (EMB, LAYERS * EMB), BF16)
    p_rootb = par("rootb", (1, LAYERS * EMB), BF16)
    p_ee = par("ee_all", (LAYERS, P, T * EMB), BF16)
    p_bnS = par("bnS", (EMB, LAYERS), F32)
    p_bnB = par("bnB", (EMB, LAYERS), F32)
    p_headW = par("headWp", (EMB, TASKS), BF16)
    p_iota = par("iota", (P, P), BF16)
    p_iden = par("iden", (P, P), BF16)
    p_ones1 = par("ones1", (1, P), BF16)
    p_src = par("src_pos", (P, T), I32)
    p_norm = par("norm_st", (P, T), BF16)
    p_dstl = par("dstl_st", (P, T), BF16)
    p_dinv = par("dinv_w", (P, nw), F32)
    p_gloc = par("glocal", (P, nw), BF16)
    p_out = nc.declare_dram_parameter("out", [nw, P, TASKS], F32, isOutput=True)

    xl_sh = [nc.dram_tensor(f"xl_sh{i}", [npad, EMB], BF16) for i in range(2)]
    xl_full = [nc.dram_tensor(f"xl_full{i}", [n_cores * npad, EMB], BF16,
                              addr_space="Shared") for i in range(2)]

    groups = [list(range(n_cores))]
    cum, blk = plan.cum, plan.blk

    with tile.TileContext(nc) as tc:
        with tc.tile_pool(name="const", bufs=1) as cpool, \
             tc.tile_pool(name="sbw", bufs=3) as sbw, \
             tc.tile_pool(name="sbg", bufs=2) as sbg, \
             tc.tile_pool(name="sbx", bufs=2) as sbx, \
             tc.tile_pool(name="psA", bufs=2, space="PSUM") as psA, \
             tc.tile_pool(name="psM", bufs=3, space="PSUM") as psM, \
             tc.tile_pool(name="psT", bufs=3, space="PSUM") as psT:

            def cload(ap, shape, dt, name):
                t = cpool.tile(list(shape), dt, tag=name)
                nc.sync.dma_start(out=t[:], in_=ap)
                return t

            iota = cload(p_iota[:, :], (P, P), BF16, "iota")
            iden = cload(p_iden[:, :], (P, P), BF16, "iden")
            ones1 = cload(p_ones1[:, :], (1, P), BF16, "ones1")
            Wl = cload(p_W[:, :], (EMB, LAYERS * EMB), BF16, "Wl")
            rootb = cload(p_rootb[:, :], (1, LAYERS * EMB), BF16, "rootb")
            bnS = cload(p_bnS[:, :], (EMB, LAYERS), F32, "bnS")
            bnB = cload(p_bnB[:, :], (EMB, LAYERS), F32, "bnB")
            headW = cload(p_headW[:, :], (EMB, TASKS), BF16, "headW")
            srcs = cload(p_src[:, :], (P, T), I32, "srcs")
            norms = cload(p_norm[:, :], (P, T), BF16, "norms")
            dstls = cload(p_dstl[:, :], (P, T), BF16, "dstls")
            dinvw = cload(p_dinv[:, :], (P, nw), F32, "dinvw")
            glocw = cload(p_gloc[:, :], (P, nw), BF16, "glocw")

            def emit_ag(l, q):
                """AllGather chunk q of xl for layer l (reads xl_sh[l%2])."""
                r0, r1 = int(cum[q]), int(cum[q + 1])
                nc.gpsimd.collective_compute(
                    "AllGather", ALU.bypass,
                    ins=[xl_sh[l % 2][r0:r1, :].opt()],
                    outs=[xl_full[l % 2][blk[q]:blk[q + 1], :].opt()],
                    replica_groups=groups)

            def xl_tail(l, w, hTs, xlsg):
                """hTs [feat,nodes] -> xl for layer l+1 into xlsg col w%GROUP."""
                xlp = psM.tile([P, EMB], F32, tag="mm")
                nc.tensor.matmul(out=xlp[:], lhsT=hTs[:],
                                 rhs=Wl[:, (l + 1) * EMB:(l + 2) * EMB],
                                 start=True, stop=False)
                nc.tensor.matmul(out=xlp[:], lhsT=ones1[:],
                                 rhs=rootb[0:1, (l + 1) * EMB:(l + 2) * EMB],
                                 start=False, stop=True)
                i = w % GROUP
                nc.vector.tensor_copy(out=xlsg[:, i * EMB:(i + 1) * EMB],
                                      in_=xlp[:])

            def head_tail(l, w, hTs):
                """Last layer: head matmul + graph pooling, DMA to p_out."""
                zp = psM.tile([P, TASKS], F32, tag="mm")
                nc.tensor.matmul(out=zp[:], lhsT=hTs[:], rhs=headW[:],
                                 start=True, stop=True)
                zs = sbw.tile([P, TASKS], BF16, tag="zs")
                nc.scalar.activation(out=zs[:], in_=zp[:], func=AF.Copy)
                selg = sbw.tile([P, P], BF16, tag="selg")
                nc.vector.tensor_tensor(
                    out=selg[:],
                    in0=glocw[:, w:w + 1].to_broadcast([P, P]),
                    in1=iota[:], op=ALU.is_equal)
                pp = psM.tile([P, TASKS], F32, tag="mm")
                nc.tensor.matmul(out=pp[:], lhsT=selg[:], rhs=zs[:],
                                 start=True, stop=True)
                ps = sbw.tile([P, TASKS], F32, tag="ps")
                nc.vector.tensor_copy(out=ps[:], in_=pp[:])
                nc.sync.dma_start(out=p_out[w, :, :], in_=ps[:])

            # ---------------- encoder: xl_0 ----------------
            for g in range(ngrp):
                h0g = sbx.tile([P, GROUP * EMB], BF16, tag="h0g")
                nc.sync.dma_start(
                    out=h0g[:], in_=p_h0T[:, g * GROUP * P:(g + 1) * GROUP * P])
                xlsg = sbx.tile([P, GROUP * EMB], BF16, tag="xlsg")
                for i in range(GROUP):
                    xlp = psM.tile([P, EMB], F32, tag="mm")
                    nc.tensor.matmul(out=xlp[:],
                                     lhsT=h0g[:, i * P:(i + 1) * P],
                                     rhs=Wl[:, 0:EMB], start=True, stop=False)
                    nc.tensor.matmul(out=xlp[:], lhsT=ones1[:],
                                     rhs=rootb[0:1, 0:EMB],
                                     start=False, stop=True)
                    nc.vector.tensor_copy(out=xlsg[:, i * EMB:(i + 1) * EMB],
                                          in_=xlp[:])
                nc.sync.dma_start(
                    out=xl_sh[0][g * GROUP * P:(g + 1) * GROUP * P, :]
                    .rearrange("(a p) e -> p a e", p=P),
                    in_=xlsg[:].rearrange("p (a e) -> p a e", e=EMB))
                if g + 1 in np.cumsum(grp_per_chunk).tolist():
                    emit_ag(0, int(np.searchsorted(
                        np.cumsum(grp_per_chunk), g + 1, side="left")))

            # ---------------- layers ----------------
            for l in range(LAYERS):
                xlf = xl_full[l % 2]
                cur = xl_sh[l % 2]
                gbuf = msgb = selb = None
                t_idx = 0
                pend = []  # (w, hnew, xlsg) tails deferred one window

                def emit_tail(l, w, hnew, xlsg):
                    hTp = psT.tile([P, EMB], BF16, tag="mmT")
                    nc.tensor.transpose(out=hTp[:], in_=hnew[:],
                                        identity=iden[:])
                    hTs = sbw.tile([P, EMB], BF16, tag="hTs")
                    if l < LAYERS - 1:
                        nc.scalar.activation(
                            out=hTs[:], in_=hTp[:], func=AF.Relu,
                            scale=bnS[:, l:l + 1], bias=bnB[:, l:l + 1])
                        xl_tail(l, w, hTs, xlsg)
                    else:
                        nc.scalar.activation(out=hTs[:], in_=hTp[:],
                                             func=AF.Copy)
                        head_tail(l, w, hTs)

                def emit_chunk(j, l=l, xlf=xlf):
                    nonlocal gbuf, msgb, selb
                    t0 = j * kg
                    gbuf = sbg.tile([P, kg * EMB], BF16, tag="gbuf")
                    for i in range(kg):
                        nc.gpsimd.indirect_dma_start(
                            out=gbuf[:, i * EMB:(i + 1) * EMB],
                            out_offset=None,
                            in_=xlf[:, :],
                            in_offset=IndirectOffsetOnAxis(
                                ap=srcs[:, t0 + i:t0 + i + 1], axis=0))
                    eeb = sbg.tile([P, kg * EMB], BF16, tag="eeb")
                    nc.sync.dma_start(
                        out=eeb[:], in_=p_ee[l, :, t0 * EMB:(t0 + kg) * EMB])
                    selb = sbg.tile([P, kg * P], BF16, tag="selb")
                    nc.vector.tensor_tensor(
                        out=selb[:].rearrange("p (k c) -> p k c", k=kg),
                        in0=dstls[:, t0:t0 + kg].unsqueeze(2)
                        .to_broadcast([P, kg, P]),
                        in1=iota[:].unsqueeze(1).to_broadcast([P, kg, P]),
                        op=ALU.is_equal)
                    nc.vector.tensor_tensor(
                        out=selb[:].rearrange("p (k c) -> p k c", k=kg),
                        in0=selb[:].rearrange("p (k c) -> p k c", k=kg),
                        in1=norms[:, t0:t0 + kg].unsqueeze(2)
                        .to_broadcast([P, kg, P]),
                        op=ALU.mult)
                    msgb = sbg.tile([P, kg * EMB], BF16, tag="msgb")
                    nc.vector.tensor_tensor(out=msgb[:], in0=gbuf[:],
                                            in1=eeb[:], op=ALU.add)
                    nc.scalar.activation(out=msgb[:], in_=msgb[:],
                                         func=AF.Relu)

                for g in range(ngrp):
                    xlog = sbx.tile([P, GROUP * EMB], BF16, tag="xlog")
                    nc.sync.dma_start(
                        out=xlog[:].rearrange("p (a e) -> p a e", e=EMB),
                        in_=cur[g * GROUP * P:(g + 1) * GROUP * P, :]
                        .rearrange("(a p) e -> p a e", p=P))
                    xlsg = None
                    if l < LAYERS - 1:
                        xlsg = sbx.tile([P, GROUP * EMB], BF16, tag="xlsg")
                    for i in range(GROUP):
                        w = g * GROUP + i
                        aggp = psA.tile([P, EMB], F32, tag="agg")
                        kw = plan.K_w[w]
                        for k in range(kw):
                            t = t_idx + k
                            if t % kg == 0:
                                emit_chunk(t // kg)
                            base = (t % kg)
                            nc.tensor.matmul(
                                out=aggp[:],
                                lhsT=selb[:, base * P:(base + 1) * P],
                                rhs=msgb[:, base * EMB:(base + 1) * EMB],
                                start=(k == 0), stop=(k == kw - 1))
                        t_idx += kw

                        sf = sbw.tile([P, EMB], F32, tag="sf")
                        nc.scalar.activation(
                            out=sf[:], in_=xlog[:, i * EMB:(i + 1) * EMB],
                            func=AF.Relu, scale=dinvw[:, w:w + 1])
                        hnew = sbw.tile([P, EMB], BF16, tag="hnew")
                        nc.vector.tensor_tensor(out=hnew[:], in0=sf[:],
                                                in1=aggp[:], op=ALU.add)
                        pend.append((w, hnew, xlsg))
                        if len(pend) > 1:
                            emit_tail(l, *pend.pop(0))
                    while pend:
                        emit_tail(l, *pend.pop(0))
                    if l < LAYERS - 1:
                        nxt = xl_sh[(l + 1) % 2]
                        nc.sync.dma_start(
                            out=nxt[g * GROUP * P:(g + 1) * GROUP * P, :]
                            .rearrange("(a p) e -> p a e", p=P),
                            in_=xlsg[:].rearrange("p (a e) -> p a e", e=EMB))
                        if g + 1 in np.cumsum(grp_per_chunk).tolist():
                            emit_ag(l + 1, int(np.searchsorted(
                                np.cumsum(grp_per_chunk), g + 1, side="left")))

    nc.finalize()
    return nc


_CACHE = {}


def kernel(**inputs):
    key = "prog"
    if key not in _CACHE:
        plan = Plan(inputs)
        warr = plan.weight_arrays(inputs)
        nc = build_program(plan)
        _CACHE[key] = (plan, nc)
    else:
        plan, nc = _CACHE[key]
        warr = plan.weight_arrays(inputs)

    in_maps = []
    for c in range(N_CORES):
        m = dict(warr)
        m["h0T"] = plan.h0T[c]
        m["ee_all"] = plan.build_ee(c)
        m["src_pos"] = plan.src_pos[c]
        m["norm_st"] = plan.norm_st[c]
        m["dstl_st"] = plan.dstl_st[c]
        m["dinv_w"] = plan.dinv_w[c]
        m["glocal"] = plan.glocal[c]
        in_maps.append(m)

    import os
    trace = bool(os.environ.get("BASS_GNN_TRACE"))
    if trace:
        try:
            import ntff_hook
            ntff_hook.install()
        except Exception:
            trace = False
    res = run_bass_kernel_spmd(nc, in_maps, list(range(N_CORES)),
                               trace=trace)
    global _LAST_EXEC_NS, _LAST_RES
    _LAST_EXEC_NS = res.exec_time_ns
    _LAST_RES = res
    blocks = [np.asarray(r["out"], np.float32) for r in res.results]
    return plan.postprocess(blocks)
